# revision 33
# baseline (speedup 1.0000x reference)
"""ConceptNet kernel for 8 Trainium2 NeuronCores.

Problem (nn_ConceptNet): given train embeddings X [B=8192, D=768], concept
matrix C [D, NC=64], the train-embedding bank T [D, N=100000], and a linear
head (W [10, D], b [10]):
  - orig_pred = X @ W.T + b
  - y_pred    = X @ (C (C^T C)^-1 C^T) @ W.T + b
  - per-concept 50-NN over T by euclidean distance -> L_sparse_1
  - gram-derived scalars (L_sparse_2, norm_metrics, similarity_penalty)

Sharding: T is split along N across the 8 cores (12500 columns each, padded
to 12512 with a sentinel high-norm column); X is split along B (1024 rows
each). concept / W / folded projection weights are replicated.

Each core computes, fully on device:
  score[c, j] = 2 * (C^T T)[c, j] - ||T_j||^2        (= c2 - d2, so ranking
                by descending score == ascending distance; c2 is a per-row
                constant and drops out of the ranking)
  via PE matmuls (f32r fast path), squares on ACT, then an exact per-row
  (per-concept) top-56 extraction on DVE:
    1) segment max over segments of 16 -> segmax [64, 782]
    2) 7 rounds of (max8 / max_index / match_replace) on segmax -> the top-56
       segments per row.  The top-50 elements of a row provably live inside
       its top-50 segments, so 56 segments form an exact superset.
    3) one indirect DMA gathers those 56*16 = 896 candidate elements from a
       DRAM dump of the score matrix
    4) 7 more rounds of (max8 / max_index / match_replace) on the gathered
       array -> top-56 element values + their gathered positions
  plus the two [10, 1024] prediction matmuls against pre-transposed X.

Host glue: builds the tiny folded matrices (gram, inverse, C G^-1 C^T W^T),
merges per-core top-56 candidates into the exact global top-50 per concept,
computes L_sparse_1 from the selected T columns in fp32, and assembles the
reference-shaped output tuple.
"""

import os
import numpy as np

import concourse.bass as bass
import concourse.bacc as bacc
import concourse.mybir as mybir
from concourse.bass import IndirectOffsetOnAxis
from concourse.bass_utils import run_bass_kernel_spmd
from concourse.tile import TileContext
from contextlib import ExitStack

# ---------------------------------------------------------------- constants
D = 768          # embedding dim
N = 100000       # train bank size
NC = 64          # number of concepts
B = 8192         # batch
C_CLS = 10       # classes
K = 50           # neighbors
M = 8            # cores
NLOC = N // M            # real columns per core (12500)
SEG = 64                 # top-k segment width (64 f32 = 256B dma_gather run)
ROUNDS = 7               # 7 * 8 = 56 >= K survivors per stage
NSEL = ROUNDS * 8        # 56
NLOC_PAD = 12544         # 196 segments * 64
NSEG = NLOC_PAD // SEG   # 196
CHUNK = 448              # matmul free-dim chunk (7 segments), >=256 for f32r
NCH = NLOC_PAD // CHUNK  # 28
NSEG_CH = CHUNK // SEG   # 7
KT = D // 128            # 6 contraction tiles
BB = B // M              # batch rows per core (1024)
PCH = 512                # prediction matmul free chunk (psum bank limit)
NEG = -3.0e38            # "removed" marker for match_replace
SENT = 1.0e6             # sentinel value for padded T columns -> score ~ -1e12

SCORE_F32R = True        # f32r (fast fp32) for the distance matmuls
PREDS_F32R = True        # f32r for the prediction matmuls

_f32 = mybir.dt.float32
_f32r = mybir.dt.float32r
_u16 = mybir.dt.uint16
_u32 = mybir.dt.uint32


def build_program(nloc_pad=NLOC_PAD, chunk=CHUNK, bb=BB, n_devices=M,
                  score_f32r=SCORE_F32R, preds_f32r=PREDS_F32R):
    """Build the single-core Bass/Tile program (run SPMD on all cores)."""
    nch = nloc_pad // chunk
    nseg = nloc_pad // SEG
    nseg_ch = chunk // SEG
    assert nloc_pad % chunk == 0 and chunk % SEG == 0
    sdt = _f32r if score_f32r else _f32
    pdt = _f32r if preds_f32r else _f32

    nc = bacc.Bacc("TRN2", target_bir_lowering=False, debug=False,
                   num_devices=n_devices)

    # tensors feeding an f32r matmul must be f32r end-to-end (walrus
    # verifier: "consumed by FP32r matmult but not rounded to FP32r").
    # All replicated operands live in ONE packed tensor loaded by ONE DMA:
    # a matmul may only carry a couple of sem waits (S3_LW struct limit in
    # walrus codegen), so every weight/activation dep must collapse onto a
    # single DMA sem lane the PE observes once at the start.
    # layout per partition: [c2 6*64 | ng 64 | wt 6*10 | pm 6*10 | xt 6*bb]
    O_NG = KT * NC                 # 384
    O_WT = O_NG + NC               # 448
    O_PM = O_WT + KT * C_CLS       # 508
    O_XT = O_PM + KT * C_CLS       # 568
    CST_W = O_XT + KT * bb
    tsl = nc.dram_tensor("tsl", [D, nloc_pad], sdt, kind="ExternalInput").ap()
    cst = nc.dram_tensor("cst", [128, CST_W], sdt, kind="ExternalInput").ap()

    vals_o = nc.dram_tensor("vals", [NC, NSEL], _f32, kind="ExternalOutput").ap()
    pos_o = nc.dram_tensor("pos", [NC, NSEL], _u16, kind="ExternalOutput").ap()
    seg_o = nc.dram_tensor("seg", [NC, NSEL], _u16, kind="ExternalOutput").ap()
    opred_o = nc.dram_tensor("opredT", [C_CLS, bb], _f32, kind="ExternalOutput").ap()
    ypred_o = nc.dram_tensor("ypredT", [C_CLS, bb], _f32, kind="ExternalOutput").ap()

    # scratch DRAM holding the full score matrix as [row*seg, 16] runs
    sc_dram = nc.dram_tensor("score_scratch", [NC * nseg, SEG], _f32).ap()

    with TileContext(nc) as tc, ExitStack() as ctx:
        consts = ctx.enter_context(tc.tile_pool(name="consts", bufs=1))
        tpool = ctx.enter_context(tc.tile_pool(name="tchunk", bufs=4))
        sqpool = ctx.enter_context(tc.tile_pool(name="sqchunk", bufs=4))
        pspool = ctx.enter_context(tc.tile_pool(name="ps", bufs=5, space="PSUM"))
        pppool = ctx.enter_context(tc.tile_pool(name="psp", bufs=2, space="PSUM"))
        wpool = ctx.enter_context(tc.tile_pool(name="warm", bufs=1, space="PSUM"))
        topk = ctx.enter_context(tc.tile_pool(name="topk", bufs=1))
        small = ctx.enter_context(tc.tile_pool(name="small", bufs=2))

        # ---- replicated constants (single tile, single DMA, single sem)
        cst_sb = consts.tile([128, CST_W], sdt)
        nc.sync.dma_start(cst_sb[:], cst)

        def c2_v(kt):
            return cst_sb[:, kt * NC:(kt + 1) * NC]

        ng_v = cst_sb[:, O_NG:O_NG + NC]

        def wt_v(kt):
            return cst_sb[:, O_WT + kt * C_CLS:O_WT + (kt + 1) * C_CLS]

        def pm_v(kt):
            return cst_sb[:, O_PM + kt * C_CLS:O_PM + (kt + 1) * C_CLS]

        def xt_v(kt, lo, hi):
            base = O_XT + kt * bb
            return cst_sb[:, base + lo:base + hi]

        segmax = topk.tile([NC, nseg], _f32)
        score_sb = topk.tile([NC, nloc_pad], _f32)
        tsl_v = tsl.rearrange("(k p) n -> p k n", p=128)
        # [NC*nseg, 16] viewed as [NC, nseg*16] rows for the chunk dumps
        sc_rows = sc_dram.rearrange("(r s) e -> r (s e)", r=NC)

        # warmup matmul reading only cst: PE observes the cst DMA sem here,
        # so the real matmuls never carry more than one sync wait (the
        # walrus S3_LW struct fits only one)
        wps = wpool.tile([2, NC], _f32)
        nc.tensor.matmul(wps[:], lhsT=c2_v(0)[:, 0:2], rhs=c2_v(1),
                         start=True, stop=True)

        # ---- score matrix: 2*C^T T - ||T||^2, chunked along N
        for ci in range(nch):
            c0 = ci * chunk
            tt = tpool.tile([128, KT, chunk], sdt)
            # issue from ACT: the slot-release dep on ACT (the square) is
            # then same-queue program order, leaving one PE wait — DMA
            # instructions fit only one sync wait in walrus codegen
            nc.scalar.dma_start(tt[:], tsl_v[:, :, c0:c0 + chunk])
            sq = sqpool.tile([128, KT, chunk], sdt)
            nc.scalar.square(sq[:].rearrange("p k n -> p (k n)"),
                             tt[:].rearrange("p k n -> p (k n)"))
            ps = pspool.tile([NC, chunk], _f32, tag="ps")
            for kt in range(KT):
                nc.tensor.matmul(ps[:], lhsT=c2_v(kt),
                                 rhs=tt[:, kt, :],
                                 start=(kt == 0), stop=False)
            for kt in range(KT):
                nc.tensor.matmul(ps[:], lhsT=ng_v,
                                 rhs=sq[:, kt, :],
                                 start=False, stop=(kt == KT - 1))
            sb = score_sb[:, c0:c0 + chunk]
            # ACT (not DVE) copy: the psum-slot release then lives on ACT,
            # which the PE already observes via each chunk's square dep
            nc.scalar.copy(sb, ps[:])
            nc.vector.tensor_reduce(
                segmax[:, ci * nseg_ch:(ci + 1) * nseg_ch],
                sb.rearrange("c (s e) -> c s e", e=SEG),
                axis=mybir.AxisListType.X, op=mybir.AluOpType.max)

        # one dump DMA (not per-chunk): the indirect gather below can then
        # depend on a single DMA-completion sem instead of 8 rotating lanes
        nc.sync.dma_start(sc_rows[:, :], score_sb[:])

        # ---- stage 1: top-56 segments per row
        seg_sb = topk.tile([NC, NSEL], _u16)
        for r in range(ROUNDS):
            mv8 = small.tile([NC, 8], _f32, tag="mv8")
            nc.vector.max(out=mv8[:], in_=segmax[:])
            nc.vector.max_index(out=seg_sb[:, r * 8:(r + 1) * 8],
                                in_max=mv8[:], in_values=segmax[:])
            nc.vector.match_replace(out=segmax[:], in_to_replace=mv8[:],
                                    in_values=segmax[:], imm_value=NEG)

        # ---- gather the top-56 segments ([NSEL, 64] runs) of each row via
        # dma_gather.  Index list position i = g*128 + r maps run g of row r
        # onto out partition r; positions live wrapped at idxs[i%16, i//16]
        # and replicated across the 8 Q7 cores.  w[16c+m, g*8+u] =
        # 196*(16u+m) + segidx[16u+m, g]  (u >= NC/16 slots are dummy 0s).
        rowbase = consts.tile([NC, NSEL], _u16)
        nc.gpsimd.iota(rowbase[:], pattern=[[0, NSEL]], base=0,
                       channel_multiplier=nseg)
        seg16b = small.tile([NC, NSEL], _u16, tag="seg16b")
        nc.vector.tensor_add(seg16b[:], seg_sb[:], rowbase[:])

        WQ = (NSEL * 128) // 16                    # 448 wrapped columns
        ublk = NC // 16
        wseg_dram = nc.dram_tensor("wseg_scratch", [16, WQ], _u16).ap()
        # zero the wrapped scratch first (u-slots >= ublk stay 0 = dummy idx)
        wz = small.tile([16, WQ], _u16, tag="wz")
        nc.vector.memset(wz[:], 0)
        nc.sync.dma_start(wseg_dram, wz[:])
        # dump seg16b [64, 56] into wrapped layout: wseg[m, g*8+u] =
        # seg16b[16u+m, g]; both sides iterate (u, m, g)
        nc.sync.dma_start(
            wseg_dram.rearrange("m (g u) -> u m g", u=8)[0:ublk, :, :],
            seg16b[:])
        w_sb = topk.tile([128, WQ], _u16)
        # one broadcast load: every 16-partition group gets the same rows
        nc.sync.dma_start(
            w_sb[:],
            wseg_dram.unsqueeze(0).to_broadcast([8, 16, WQ]))

        # the Q7 gather ucode falls over somewhere above 1024 indices per
        # call on this runtime — split into 1024-index calls (slots 8k..8k+8)
        gath = topk.tile([128, NSEL, SEG], _f32)
        for k in range(NSEL // 8):
            nc.gpsimd.dma_gather(
                out_ap=gath[:, 8 * k:8 * (k + 1), :],
                in_ap=sc_dram,
                idxs_ap=w_sb[:, 64 * k:64 * (k + 1)].bitcast(mybir.dt.int16),
                num_idxs=1024,
                num_idxs_reg=1024,
                elem_size=SEG,
            )

        # ---- stage 2: top-56 elements of the gathered candidates
        gv = gath[0:NC, :, :].rearrange("c j e -> c (j e)")
        vals_sb = topk.tile([NC, NSEL], _f32)
        pos_sb = topk.tile([NC, NSEL], _u16)
        for r in range(ROUNDS):
            gv8 = small.tile([NC, 8], _f32, tag="gv8")
            nc.vector.max(out=gv8[:], in_=gv)
            nc.vector.tensor_copy(vals_sb[:, r * 8:(r + 1) * 8], gv8[:])
            nc.vector.max_index(out=pos_sb[:, r * 8:(r + 1) * 8],
                                in_max=gv8[:], in_values=gv)
            nc.vector.match_replace(out=gv, in_to_replace=gv8[:],
                                    in_values=gv, imm_value=NEG)

        nc.sync.dma_start(vals_o, vals_sb[:])
        nc.sync.dma_start(pos_o, pos_sb[:])
        nc.sync.dma_start(seg_o, seg_sb[:])

        # ---- predictions: opredT = W X^T, ypredT = (C G^-1 C^T W^T)^T X^T
        for w_v, out_ap in ((wt_v, opred_o), (pm_v, ypred_o)):
            for h in range(bb // PCH):
                pso = pppool.tile([C_CLS, PCH], _f32, tag="pso")
                for kt in range(KT):
                    nc.tensor.matmul(
                        pso[:], lhsT=w_v(kt),
                        rhs=xt_v(kt, h * PCH, (h + 1) * PCH),
                        start=(kt == 0), stop=(kt == KT - 1))
                ob = small.tile([C_CLS, PCH], _f32, tag="predout")
                nc.scalar.copy(ob[:], pso[:])
                nc.sync.dma_start(out_ap[:, h * PCH:(h + 1) * PCH], ob[:])

    nc.compile()
    return nc


_CACHED_NC = None


def _get_program():
    global _CACHED_NC
    if _CACHED_NC is None:
        _CACHED_NC = build_program()
    return _CACHED_NC


def pack_cst(C2, W_t, P2M2, XT, bb):
    """Pack replicated operands into the [128, CST_W] constant tensor.

    Column layout per partition p (see build_program):
      [c2: kt*64+c] [ng: 64] [wt: kt*10+j] [pm: kt*10+j] [xt: kt*bb+b]
    where row kt*128+p of the [768, *] operand lands in partition p.
    """
    def fold(a, width):
        # [768, width] -> [128, KT*width] with a[kt*128+p, j] at [p, kt*width+j]
        return a.reshape(KT, 128, width).transpose(1, 0, 2).reshape(128, KT * width)

    cst = np.concatenate([
        fold(C2, NC),
        np.full((128, NC), -1.0, np.float32),
        fold(W_t, C_CLS),
        fold(P2M2, C_CLS),
        fold(XT, bb),
    ], axis=1)
    return np.ascontiguousarray(cst, dtype=np.float32)


def _host_prep(train_embedding, concept, train_embeddings_T, W):
    """Build per-core input maps + the folded matrices used on host."""
    X = np.ascontiguousarray(np.asarray(train_embedding, dtype=np.float32))
    C = np.ascontiguousarray(np.asarray(concept, dtype=np.float32))
    T = np.asarray(train_embeddings_T, dtype=np.float32)
    W = np.asarray(W, dtype=np.float32)

    gram = C.T @ C                                        # [NC, NC]
    inv = np.linalg.inv(gram.astype(np.float64)).astype(np.float32)
    p2m2 = np.ascontiguousarray(C @ (inv @ (C.T @ W.T)))  # [D, 10]
    wt = np.ascontiguousarray(W.T)                        # [D, 10]
    c2 = np.ascontiguousarray(2.0 * C)                    # [D, NC]

    pad_col = np.zeros((D, 1), np.float32)
    pad_col[0, 0] = SENT
    npad_half = NLOC_PAD - NLOC
    in_maps = []
    for c in range(M):
        slab = T[:, c * NLOC:(c + 1) * NLOC]
        slab = np.concatenate(
            [slab, np.broadcast_to(pad_col, (D, npad_half))], axis=1)
        xts = np.ascontiguousarray(X[c * BB:(c + 1) * BB, :].T)
        in_maps.append({
            "tsl": np.ascontiguousarray(slab),
            "cst": pack_cst(c2, wt, p2m2, xts, BB),
        })
    return in_maps, gram, C, T


def _host_merge(results, gram, C, T, b, topk):
    """Merge per-core outputs into the reference-shaped tuple."""
    k = int(topk)
    b = np.asarray(b, dtype=np.float32)

    # global top-k per concept from the per-core candidate lists
    all_vals = []
    all_gidx = []
    for c, res in enumerate(results):
        vals = res["vals"]                        # [NC, 56] f32
        pos = res["pos"].astype(np.int64)         # [NC, 56] positions in gath
        seg = res["seg"].astype(np.int64)         # [NC, 56] segment ids
        rows = np.arange(NC)[:, None]
        eidx = seg[rows, pos // SEG] * SEG + pos % SEG     # local column
        valid = eidx < NLOC
        gidx = c * NLOC + np.minimum(eidx, NLOC - 1)
        v = np.where(valid, vals, -np.inf)
        all_vals.append(v)
        all_gidx.append(gidx)
    av = np.concatenate(all_vals, axis=1)         # [NC, 8*56]
    ag = np.concatenate(all_gidx, axis=1)

    idx = np.empty((NC, k), dtype=np.int64)
    for r in range(NC):
        g, first = np.unique(ag[r], return_index=True)
        v = av[r][first]
        order = np.lexsort((g, -v))[:k]
        idx[r] = g[order]

    knn = T[:, idx]                                # [D, NC, K]
    dot = np.einsum("dc,dck->c", C, knn) / np.float32(k)
    L1 = np.float32(dot.mean())

    eye = np.eye(NC, dtype=np.float32)
    L2 = np.float32(np.mean(gram * (1.0 - eye)))
    nm = np.float32(np.mean(gram * eye))
    sp = np.float32(np.mean(np.abs(gram - eye)))

    opred = np.concatenate([res["opredT"].T for res in results], axis=0) + b
    ypred = np.concatenate([res["ypredT"].T for res in results], axis=0) + b
    return (np.ascontiguousarray(opred.astype(np.float32)),
            np.ascontiguousarray(ypred.astype(np.float32)),
            L1, L2, nm, sp)


def kernel(train_embedding, h_x, concept, train_embeddings_T, W, b, topk):
    in_maps, gram, C, T = _host_prep(train_embedding, concept,
                                     train_embeddings_T, W)
    nc = _get_program()
    res = run_bass_kernel_spmd(nc, in_maps, core_ids=list(range(M)))
    return _host_merge(res.results, gram, C, T, b, topk)
